# revision 9
# baseline (speedup 1.0000x reference)
"""Contrastive cosine-similarity MSE loss kernel for Trainium2 (8 cores).

Math (reference): scores_n = <a_n, b_n> / (||a_n|| * ||b_n||);
loss = mean((scores - labels)^2) over N=8192 rows, D=1024.

Sharding: data-parallel over rows. Core c handles rows [c*1024, (c+1)*1024).
Per core, 4 tiles of [128p x 2048] where partition p holds two consecutive
rows (8KB contiguous DRAM run per partition -> large DMA packets):
  - VectorE: fused multiply+row-sum  -> dots        (scalar_tensor_tensor)
  - ScalarE: fused square+row-sum    -> ||a||^2     (activation Square)
  - VectorE: fused square+row-sum    -> ||b||^2     (scalar_tensor_tensor)
Tail on [128, 8] stats: score = dots * 1/sqrt(na*nb); partial[p] =
sum_c (score - label)^2. Host sums the 8x128 partials and divides by N.

Row mapping: stats col c = 2*i + j (tile i, half j); stats[p, c] is local
row i*256 + 2*p + j. Labels are pre-permuted on the host to match.
"""

import numpy as np

import concourse.bacc as bacc
import concourse.bass as bass
import concourse.tile as tile
from concourse import mybir
from concourse.bass_utils import run_bass_kernel_spmd

N, D = 8192, 1024
N_CORES = 8
ROWS = N // N_CORES  # rows per core
P = 128  # SBUF partitions
RPT = 256  # rows per tile (2 per partition)
NTILES = ROWS // RPT  # 4
NC_ = 2 * NTILES  # stats columns

_cache = {}


def _hbm_tile_ap(t, tile_idx):
    """AP over DRAM matrix t ([ROWS, D]) for tile `tile_idx`:
    [128 partitions, 2048 free], partition p = rows (tile_idx*256 + 2p, +1),
    free dim contiguous 8KB."""
    base = tile_idx * RPT * D
    return bass.AP(tensor=t, offset=base, ap=[[2 * D, P], [1, 2 * D]])


def _build():
    nc = bacc.Bacc("TRN2", target_bir_lowering=False, debug=False)

    f32 = mybir.dt.float32
    a = nc.dram_tensor("a", [ROWS, D], f32, kind="ExternalInput")
    b = nc.dram_tensor("b", [ROWS, D], f32, kind="ExternalInput")
    # labels pre-permuted on host to [P, NC_] (see module docstring)
    lab = nc.dram_tensor("lab_t", [P, NC_], f32, kind="ExternalInput")
    out = nc.dram_tensor("out", [P, 1], f32, kind="ExternalOutput")

    with tile.TileContext(nc) as tc:
        with (
            tc.tile_pool(name="io", bufs=3) as io_pool,
            tc.tile_pool(name="scratch", bufs=2) as scr_pool,
            tc.tile_pool(name="stats", bufs=1) as st_pool,
        ):
            dots = st_pool.tile([P, NC_], f32)
            na = st_pool.tile([P, NC_], f32)
            nb = st_pool.tile([P, NC_], f32)
            labt = st_pool.tile([P, NC_], f32)
            nc.sync.dma_start(out=labt, in_=lab[:, :])

            for t in range(NTILES):
                at = io_pool.tile([P, 2 * D], f32, tag="a")
                bt = io_pool.tile([P, 2 * D], f32, tag="b")
                # split by partition halves: 2 transfers per tile per matrix
                a_src = _hbm_tile_ap(a, t)
                b_src = _hbm_tile_ap(b, t)
                nc.sync.dma_start(out=at[0:64, :], in_=a_src[0:64, :])
                nc.sync.dma_start(out=at[64:128, :], in_=a_src[64:128, :])
                nc.sync.dma_start(out=bt[0:64, :], in_=b_src[0:64, :])
                nc.sync.dma_start(out=bt[64:128, :], in_=b_src[64:128, :])

                for j in range(2):
                    c = 2 * t + j
                    asl = at[:, j * D : (j + 1) * D]
                    bsl = bt[:, j * D : (j + 1) * D]
                    sd = scr_pool.tile([P, D], f32, tag="sdve")
                    sa = scr_pool.tile([P, D], f32, tag="sact")
                    sb = scr_pool.tile([P, D], f32, tag="sdve")
                    # dots[:, c] = sum_d a*b  (VectorE, one fused pass)
                    nc.vector.scalar_tensor_tensor(
                        out=sd,
                        in0=asl,
                        scalar=1.0,
                        in1=bsl,
                        op0=mybir.AluOpType.mult,
                        op1=mybir.AluOpType.mult,
                        accum_out=dots[:, c : c + 1],
                    )
                    # na[:, c] = sum_d a^2 (ScalarE)
                    nc.scalar.activation(
                        out=sa,
                        in_=asl,
                        func=mybir.ActivationFunctionType.Square,
                        accum_out=na[:, c : c + 1],
                    )
                    # nb[:, c] = sum_d b^2 (VectorE)
                    nc.vector.scalar_tensor_tensor(
                        out=sb,
                        in0=bsl,
                        scalar=1.0,
                        in1=bsl,
                        op0=mybir.AluOpType.mult,
                        op1=mybir.AluOpType.mult,
                        accum_out=nb[:, c : c + 1],
                    )

            # Tail on [P, NC_] stats (tiny).
            prod = st_pool.tile([P, NC_], f32)
            nc.vector.tensor_mul(prod, na, nb)
            nc.scalar.sqrt(prod, prod)
            rs = st_pool.tile([P, NC_], f32)
            nc.vector.reciprocal(rs, prod)
            score = st_pool.tile([P, NC_], f32)
            nc.vector.tensor_mul(score, dots, rs)
            diff = st_pool.tile([P, NC_], f32)
            nc.vector.tensor_sub(diff, score, labt)
            sqd = st_pool.tile([P, NC_], f32)
            partial = st_pool.tile([P, 1], f32)
            nc.vector.scalar_tensor_tensor(
                out=sqd,
                in0=diff,
                scalar=1.0,
                in1=diff,
                op0=mybir.AluOpType.mult,
                op1=mybir.AluOpType.mult,
                accum_out=partial,
            )
            nc.sync.dma_start(out=out[:, :], in_=partial)

    nc.compile()
    return nc


def _label_perm(lab_core):
    """[ROWS] -> [P, NC_] matching stats layout: col 2i+j, partition p
    holds local row i*256 + 2p + j."""
    return np.ascontiguousarray(
        lab_core.reshape(NTILES, P, 2).transpose(1, 0, 2).reshape(P, NC_)
    )


def kernel(issues_1_geb, issues_2_geb, labels):
    if "nc" not in _cache:
        _cache["nc"] = _build()
    nc = _cache["nc"]

    a = np.ascontiguousarray(issues_1_geb, dtype=np.float32)
    b = np.ascontiguousarray(issues_2_geb, dtype=np.float32)
    lab = np.ascontiguousarray(labels, dtype=np.float32)

    in_maps = []
    for c in range(N_CORES):
        sl = slice(c * ROWS, (c + 1) * ROWS)
        in_maps.append(
            {
                "a": np.ascontiguousarray(a[sl]),
                "b": np.ascontiguousarray(b[sl]),
                "lab_t": _label_perm(lab[sl]),
            }
        )

    res = run_bass_kernel_spmd(nc, in_maps, core_ids=list(range(N_CORES)))
    total = np.float64(0.0)
    for r in res.results:
        total += np.float64(r["out"].sum(dtype=np.float64))
    return np.array(total / N, dtype=np.float32)


# revision 10
# speedup vs baseline: 1.3007x; 1.3007x over previous
"""Contrastive cosine-similarity MSE loss kernel for Trainium2 (8 cores).

Math (reference): scores_n = <a_n, b_n> / (||a_n|| * ||b_n||);
loss = mean((scores - labels)^2) over N=8192 rows, D=1024.

Sharding: data-parallel over rows. Core c handles rows [c*1024, (c+1)*1024).
Per core, 8 tiles of [128 rows x 1024 cols]:
  - VectorE: fused multiply+row-sum  -> dots        (scalar_tensor_tensor)
  - ScalarE: fused square+row-sum    -> ||a||^2     (activation Square)
  - VectorE: fused square+row-sum    -> ||b||^2     (scalar_tensor_tensor)
The mandatory full-size elementwise outputs go to PSUM so their writes
don't contend with the DMA input stream for SBUF ports.
Tail on [128, 8] stats: score = dots * 1/sqrt(na*nb); partial[p] =
sum_t (score - label)^2. Host sums the 8x128 partials and divides by N.
"""

import numpy as np

import concourse.bacc as bacc
import concourse.tile as tile
from concourse import mybir
from concourse.bass_utils import run_bass_kernel_spmd

N, D = 8192, 1024
N_CORES = 8
ROWS = N // N_CORES  # rows per core
P = 128  # SBUF partitions
NT = ROWS // P  # row-tiles per core

_cache = {}


def _build():
    nc = bacc.Bacc("TRN2", target_bir_lowering=False, debug=False)

    f32 = mybir.dt.float32
    a = nc.dram_tensor("a", [ROWS, D], f32, kind="ExternalInput")
    b = nc.dram_tensor("b", [ROWS, D], f32, kind="ExternalInput")
    # labels pre-transposed on host to [P, NT]: lab_t[p, t] = labels[t*P + p]
    lab = nc.dram_tensor("lab_t", [P, NT], f32, kind="ExternalInput")
    out = nc.dram_tensor("out", [P, 1], f32, kind="ExternalOutput")

    with tile.TileContext(nc) as tc:
        with (
            tc.tile_pool(name="io", bufs=3) as io_pool,
            tc.tile_pool(name="psum", bufs=2, space="PSUM") as ps_pool,
            tc.tile_pool(name="stats", bufs=1) as st_pool,
        ):
            dots = st_pool.tile([P, NT], f32)
            na = st_pool.tile([P, NT], f32)
            nb = st_pool.tile([P, NT], f32)
            labt = st_pool.tile([P, NT], f32)
            nc.sync.dma_start(out=labt, in_=lab[:, :])

            for t in range(NT):
                at = io_pool.tile([P, D], f32, tag="a")
                bt = io_pool.tile([P, D], f32, tag="b")
                nc.sync.dma_start(out=at, in_=a[t * P : (t + 1) * P, :])
                nc.sync.dma_start(out=bt, in_=b[t * P : (t + 1) * P, :])

                sd = ps_pool.tile([P, D], f32, tag="sdve")
                sa = ps_pool.tile([P, D], f32, tag="sact")
                sb = ps_pool.tile([P, D], f32, tag="sdve")
                # dots[:, t] = sum_d at * bt  (VectorE, one fused pass)
                nc.vector.scalar_tensor_tensor(
                    out=sd,
                    in0=at,
                    scalar=1.0,
                    in1=bt,
                    op0=mybir.AluOpType.mult,
                    op1=mybir.AluOpType.mult,
                    accum_out=dots[:, t : t + 1],
                )
                # na[:, t] = sum_d at^2 (ScalarE)
                nc.scalar.activation(
                    out=sa,
                    in_=at,
                    func=mybir.ActivationFunctionType.Square,
                    accum_out=na[:, t : t + 1],
                )
                # nb[:, t] = sum_d bt^2 (VectorE)
                nc.vector.scalar_tensor_tensor(
                    out=sb,
                    in0=bt,
                    scalar=1.0,
                    in1=bt,
                    op0=mybir.AluOpType.mult,
                    op1=mybir.AluOpType.mult,
                    accum_out=nb[:, t : t + 1],
                )

            # Tail on [P, NT] stats (tiny).
            prod = st_pool.tile([P, NT], f32)
            nc.vector.tensor_mul(prod, na, nb)
            nc.scalar.sqrt(prod, prod)
            rs = st_pool.tile([P, NT], f32)
            nc.vector.reciprocal(rs, prod)
            score = st_pool.tile([P, NT], f32)
            nc.vector.tensor_mul(score, dots, rs)
            diff = st_pool.tile([P, NT], f32)
            nc.vector.tensor_sub(diff, score, labt)
            sqd = st_pool.tile([P, NT], f32)
            partial = st_pool.tile([P, 1], f32)
            nc.vector.scalar_tensor_tensor(
                out=sqd,
                in0=diff,
                scalar=1.0,
                in1=diff,
                op0=mybir.AluOpType.mult,
                op1=mybir.AluOpType.mult,
                accum_out=partial,
            )
            nc.sync.dma_start(out=out[:, :], in_=partial)

    nc.compile()
    return nc


def _label_perm(lab_core):
    """[ROWS] -> [P, NT]: lab_t[p, t] = labels[t*P + p]."""
    return np.ascontiguousarray(lab_core.reshape(NT, P).T)


def kernel(issues_1_geb, issues_2_geb, labels):
    if "nc" not in _cache:
        _cache["nc"] = _build()
    nc = _cache["nc"]

    a = np.ascontiguousarray(issues_1_geb, dtype=np.float32)
    b = np.ascontiguousarray(issues_2_geb, dtype=np.float32)
    lab = np.ascontiguousarray(labels, dtype=np.float32)

    in_maps = []
    for c in range(N_CORES):
        sl = slice(c * ROWS, (c + 1) * ROWS)
        in_maps.append(
            {
                "a": np.ascontiguousarray(a[sl]),
                "b": np.ascontiguousarray(b[sl]),
                "lab_t": _label_perm(lab[sl]),
            }
        )

    res = run_bass_kernel_spmd(nc, in_maps, core_ids=list(range(N_CORES)))
    total = np.float64(0.0)
    for r in res.results:
        total += np.float64(r["out"].sum(dtype=np.float64))
    return np.array(total / N, dtype=np.float32)


# revision 12
# speedup vs baseline: 1.3549x; 1.0417x over previous
"""Contrastive cosine-similarity MSE loss kernel for Trainium2 (8 cores).

Math (reference): scores_n = <a_n, b_n> / (||a_n|| * ||b_n||);
loss = mean((scores - labels)^2) over N=8192 rows, D=1024.

Sharding: data-parallel over rows. Core c handles rows [c*1024, (c+1)*1024).
Per core, 8 tiles of [128 rows x 1024 cols]:
  - VectorE: fused multiply+row-sum  -> dots        (scalar_tensor_tensor)
  - ScalarE: fused square+row-sum    -> ||a||^2     (activation Square)
  - VectorE: fused square+row-sum    -> ||b||^2     (scalar_tensor_tensor)
The mandatory full-size elementwise outputs go to PSUM so their writes
don't contend with the DMA input stream for SBUF ports.
Tail on [128, 8] stats: score = dots * 1/sqrt(na*nb); partial[p] =
sum_t (score - label)^2. Host sums the 8x128 partials and divides by N.
"""

import numpy as np

import concourse.bacc as bacc
import concourse.tile as tile
from concourse import mybir
from concourse.bass_utils import run_bass_kernel_spmd
from concourse.vector_clock import ScopedClock


class _LeanTileContext(tile.TileContext):
    """TileContext with a minimal kernel epilogue.

    The stock epilogue is drain + all-engine butterfly + semaphore
    clear + second butterfly (~10us on HW). For this single-shot
    kernel we only need the drain (all DMA queues complete, so the
    output is in DRAM before the NEFF retires); engines may retire
    their streams independently."""

    def _drain_and_barrier(self, tick_clock, wait_clock):
        drain_inst = self.nc.sync.drain()
        wait_clock.add_sem_waits(
            drain_inst.ins, ScopedClock({None: tick_clock.global_clock})
        )
        popped = self.nc._tile_sem_poison_stack.pop()
        assert popped is self._sem_poison

N, D = 8192, 1024
N_CORES = 8
ROWS = N // N_CORES  # rows per core
P = 128  # SBUF partitions
NT = ROWS // P  # row-tiles per core

_cache = {}


def _build():
    nc = bacc.Bacc("TRN2", target_bir_lowering=False, debug=False)

    f32 = mybir.dt.float32
    a = nc.dram_tensor("a", [ROWS, D], f32, kind="ExternalInput")
    b = nc.dram_tensor("b", [ROWS, D], f32, kind="ExternalInput")
    # labels pre-transposed on host to [P, NT]: lab_t[p, t] = labels[t*P + p]
    lab = nc.dram_tensor("lab_t", [P, NT], f32, kind="ExternalInput")
    out = nc.dram_tensor("out", [P, 1], f32, kind="ExternalOutput")

    with _LeanTileContext(nc) as tc:
        with (
            tc.tile_pool(name="io", bufs=3) as io_pool,
            tc.tile_pool(name="psum", bufs=2, space="PSUM") as ps_pool,
            tc.tile_pool(name="stats", bufs=1) as st_pool,
        ):
            dots = st_pool.tile([P, NT], f32)
            na = st_pool.tile([P, NT], f32)
            nb = st_pool.tile([P, NT], f32)
            labt = st_pool.tile([P, NT], f32)
            nc.sync.dma_start(out=labt, in_=lab[:, :])

            for t in range(NT):
                at = io_pool.tile([P, D], f32, tag="a")
                bt = io_pool.tile([P, D], f32, tag="b")
                nc.sync.dma_start(out=at, in_=a[t * P : (t + 1) * P, :])
                nc.sync.dma_start(out=bt, in_=b[t * P : (t + 1) * P, :])

                sd = ps_pool.tile([P, D], f32, tag="sdve")
                sa = ps_pool.tile([P, D], f32, tag="sact")
                sb = ps_pool.tile([P, D], f32, tag="sdve")
                # dots[:, t] = sum_d at * bt  (VectorE, one fused pass)
                nc.vector.scalar_tensor_tensor(
                    out=sd,
                    in0=at,
                    scalar=1.0,
                    in1=bt,
                    op0=mybir.AluOpType.mult,
                    op1=mybir.AluOpType.mult,
                    accum_out=dots[:, t : t + 1],
                )
                # na[:, t] = sum_d at^2 (ScalarE)
                nc.scalar.activation(
                    out=sa,
                    in_=at,
                    func=mybir.ActivationFunctionType.Square,
                    accum_out=na[:, t : t + 1],
                )
                # nb[:, t] = sum_d bt^2 (VectorE)
                nc.vector.scalar_tensor_tensor(
                    out=sb,
                    in0=bt,
                    scalar=1.0,
                    in1=bt,
                    op0=mybir.AluOpType.mult,
                    op1=mybir.AluOpType.mult,
                    accum_out=nb[:, t : t + 1],
                )

            # Tail on [P, NT] stats (tiny).
            prod = st_pool.tile([P, NT], f32)
            nc.vector.tensor_mul(prod, na, nb)
            nc.scalar.sqrt(prod, prod)
            rs = st_pool.tile([P, NT], f32)
            nc.vector.reciprocal(rs, prod)
            score = st_pool.tile([P, NT], f32)
            nc.vector.tensor_mul(score, dots, rs)
            diff = st_pool.tile([P, NT], f32)
            nc.vector.tensor_sub(diff, score, labt)
            sqd = st_pool.tile([P, NT], f32)
            partial = st_pool.tile([P, 1], f32)
            nc.vector.scalar_tensor_tensor(
                out=sqd,
                in0=diff,
                scalar=1.0,
                in1=diff,
                op0=mybir.AluOpType.mult,
                op1=mybir.AluOpType.mult,
                accum_out=partial,
            )
            nc.sync.dma_start(out=out[:, :], in_=partial)

    nc.compile()
    return nc


def _label_perm(lab_core):
    """[ROWS] -> [P, NT]: lab_t[p, t] = labels[t*P + p]."""
    return np.ascontiguousarray(lab_core.reshape(NT, P).T)


def kernel(issues_1_geb, issues_2_geb, labels):
    if "nc" not in _cache:
        _cache["nc"] = _build()
    nc = _cache["nc"]

    a = np.ascontiguousarray(issues_1_geb, dtype=np.float32)
    b = np.ascontiguousarray(issues_2_geb, dtype=np.float32)
    lab = np.ascontiguousarray(labels, dtype=np.float32)

    in_maps = []
    for c in range(N_CORES):
        sl = slice(c * ROWS, (c + 1) * ROWS)
        in_maps.append(
            {
                "a": np.ascontiguousarray(a[sl]),
                "b": np.ascontiguousarray(b[sl]),
                "lab_t": _label_perm(lab[sl]),
            }
        )

    res = run_bass_kernel_spmd(nc, in_maps, core_ids=list(range(N_CORES)))
    total = np.float64(0.0)
    for r in res.results:
        total += np.float64(r["out"].sum(dtype=np.float64))
    return np.array(total / N, dtype=np.float32)


# revision 14
# speedup vs baseline: 1.5414x; 1.1377x over previous
"""Contrastive cosine-similarity MSE loss kernel for Trainium2 (8 cores).

Math (reference): scores_n = <a_n, b_n> / (||a_n|| * ||b_n||);
loss = mean((scores - labels)^2) over N=8192 rows, D=1024.

Sharding: data-parallel over rows. Core c handles rows [c*1024, (c+1)*1024).
Per core, 8 tiles of [128 rows x 1024 cols]:
  - VectorE: fused multiply+row-sum  -> dots        (scalar_tensor_tensor)
  - ScalarE: fused square+row-sum    -> ||a||^2     (activation Square)
  - VectorE: fused square+row-sum    -> ||b||^2     (scalar_tensor_tensor)
Labels arrive in natural [8, 128] layout (fat DMA descriptors) and are
PE-transposed to [128, 8]; the final 128-partition partial sum is reduced
to a single scalar with a ones-matmul so the output DMA is one descriptor
(a [128,1] store costs 128 4-byte descriptors and stalls the tail drain).
Host sums the 8 per-core scalars and divides by N.
"""

import numpy as np

import concourse.bacc as bacc
import concourse.tile as tile
from concourse import mybir
from concourse.bass_utils import run_bass_kernel_spmd
from concourse.masks import make_identity
from concourse.vector_clock import ScopedClock


class _LeanTileContext(tile.TileContext):
    """TileContext with a minimal kernel epilogue.

    The stock epilogue is drain + all-engine butterfly + semaphore
    clear + second butterfly. For this single-shot kernel we only need
    the drain (all DMA queues complete, so the output is in DRAM before
    the NEFF retires); engines may retire their streams independently."""

    def _drain_and_barrier(self, tick_clock, wait_clock):
        drain_inst = self.nc.sync.drain()
        wait_clock.add_sem_waits(
            drain_inst.ins, ScopedClock({None: tick_clock.global_clock})
        )
        popped = self.nc._tile_sem_poison_stack.pop()
        assert popped is self._sem_poison


N, D = 8192, 1024
N_CORES = 8
ROWS = N // N_CORES  # rows per core
P = 128  # SBUF partitions
NT = ROWS // P  # row-tiles per core

_cache = {}


def _build():
    nc = bacc.Bacc("TRN2", target_bir_lowering=False, debug=False)

    f32 = mybir.dt.float32
    a = nc.dram_tensor("a", [ROWS, D], f32, kind="ExternalInput")
    b = nc.dram_tensor("b", [ROWS, D], f32, kind="ExternalInput")
    # labels in natural layout: lab[t, p] = labels[t*P + p]
    lab = nc.dram_tensor("lab_t", [NT, P], f32, kind="ExternalInput")
    out = nc.dram_tensor("out", [1, 1], f32, kind="ExternalOutput")

    with _LeanTileContext(nc) as tc:
        with (
            tc.tile_pool(name="io", bufs=3) as io_pool,
            tc.tile_pool(name="scr", bufs=2) as scr_pool,
            tc.tile_pool(name="psa", bufs=1, space="PSUM") as psa_pool,
            tc.tile_pool(name="stats", bufs=1) as st_pool,
        ):
            dots = st_pool.tile([P, NT], f32)
            na = st_pool.tile([P, NT], f32)
            nb = st_pool.tile([P, NT], f32)

            # Labels: one fat DMA into [NT, P], then PE-transpose to [P, NT].
            lab_sb = st_pool.tile([NT, P], f32)
            nc.sync.dma_start(out=lab_sb, in_=lab[:, :])
            id8 = st_pool.tile([NT, NT], f32)
            make_identity(nc, id8)
            labt = psa_pool.tile([P, NT], f32)
            nc.tensor.transpose(labt, lab_sb, id8)

            ones = st_pool.tile([P, 1], f32)
            nc.vector.memset(ones, 1.0)

            for t in range(NT):
                at = io_pool.tile([P, D], f32, tag="a")
                bt = io_pool.tile([P, D], f32, tag="b")
                nc.sync.dma_start(out=at, in_=a[t * P : (t + 1) * P, :])
                nc.sync.dma_start(out=bt, in_=b[t * P : (t + 1) * P, :])

                sd = scr_pool.tile([P, D], f32, tag="sdve")
                sa = scr_pool.tile([P, D], f32, tag="sact")
                sb = scr_pool.tile([P, D], f32, tag="sdve")
                # dots[:, t] = sum_d at * bt  (VectorE, one fused pass)
                nc.vector.scalar_tensor_tensor(
                    out=sd,
                    in0=at,
                    scalar=1.0,
                    in1=bt,
                    op0=mybir.AluOpType.mult,
                    op1=mybir.AluOpType.mult,
                    accum_out=dots[:, t : t + 1],
                )
                # na[:, t] = sum_d at^2 (ScalarE; PSUM dest is ScalarE's fast port)
                nc.scalar.activation(
                    out=sa,
                    in_=at,
                    func=mybir.ActivationFunctionType.Square,
                    accum_out=na[:, t : t + 1],
                )
                # nb[:, t] = sum_d bt^2 (VectorE)
                nc.vector.scalar_tensor_tensor(
                    out=sb,
                    in0=bt,
                    scalar=1.0,
                    in1=bt,
                    op0=mybir.AluOpType.mult,
                    op1=mybir.AluOpType.mult,
                    accum_out=nb[:, t : t + 1],
                )

            # Tail on [P, NT] stats (tiny).
            prod = st_pool.tile([P, NT], f32)
            nc.vector.tensor_mul(prod, na, nb)
            nc.scalar.sqrt(prod, prod)
            rs = st_pool.tile([P, NT], f32)
            nc.vector.reciprocal(rs, prod)
            score = st_pool.tile([P, NT], f32)
            nc.vector.tensor_mul(score, dots, rs)
            diff = st_pool.tile([P, NT], f32)
            nc.vector.tensor_sub(diff, score, labt)
            sqd = st_pool.tile([P, NT], f32)
            partial = st_pool.tile([P, 1], f32)
            nc.vector.scalar_tensor_tensor(
                out=sqd,
                in0=diff,
                scalar=1.0,
                in1=diff,
                op0=mybir.AluOpType.mult,
                op1=mybir.AluOpType.mult,
                accum_out=partial,
            )
            # Reduce 128 partitions -> [1,1] so the output DMA is one
            # descriptor instead of 128.
            total_ps = psa_pool.tile([1, 1], f32)
            nc.tensor.matmul(total_ps, partial, ones)
            res_sb = st_pool.tile([1, 1], f32)
            nc.scalar.copy(res_sb, total_ps)
            nc.sync.dma_start(out=out[:, :], in_=res_sb)

    nc.compile()
    return nc


def _label_perm(lab_core):
    """[ROWS] -> [NT, P] natural layout."""
    return np.ascontiguousarray(lab_core.reshape(NT, P))


def kernel(issues_1_geb, issues_2_geb, labels):
    if "nc" not in _cache:
        _cache["nc"] = _build()
    nc = _cache["nc"]

    a = np.ascontiguousarray(issues_1_geb, dtype=np.float32)
    b = np.ascontiguousarray(issues_2_geb, dtype=np.float32)
    lab = np.ascontiguousarray(labels, dtype=np.float32)

    in_maps = []
    for c in range(N_CORES):
        sl = slice(c * ROWS, (c + 1) * ROWS)
        in_maps.append(
            {
                "a": np.ascontiguousarray(a[sl]),
                "b": np.ascontiguousarray(b[sl]),
                "lab_t": _label_perm(lab[sl]),
            }
        )

    res = run_bass_kernel_spmd(nc, in_maps, core_ids=list(range(N_CORES)))
    total = np.float64(0.0)
    for r in res.results:
        total += np.float64(r["out"].sum(dtype=np.float64))
    return np.array(total / N, dtype=np.float32)
